# revision 38
# baseline (speedup 1.0000x reference)
"""Trainium2 Bass kernel for nn_MultiHeadAttention (B=2, S=2048, E=1024,
H=16, D=64) on 8 NeuronCores.

Sharding: core c -> (batch b = c//4, head-group g = c%4). Each core
computes Q/K/V projections for its batch restricted to its 4 heads
(column-parallel Wq/Wk/Wv), full attention for those heads, and a
row-parallel partial fc_out (the 256 local features of Wo, bf16). The
host sums the 4 bf16 partial outputs per batch in fp32; the fc bias bo
is folded into one core per batch group via the normalized ones-row.

The ScalarE exp stream is the roofline: 128 ACTIVATEs of [128, 1024]
(~1.1us each) that everything else must hide behind.
  - q is processed in 512-wide chunks; heads in pairs (j in {0,1}) whose
    K^T/Q^T live on partition halves 0-63 / 64-127 of kt_sb[j]/qt_sb[j].
  - s-matmuls for the two heads of a pair are emitted back-to-back with
    K=64 stationaries at base partitions 0 and 64: the PE runs them
    CONCURRENTLY as row-tiles (2 MMs per N=512 slot when warm).
  - both heads' scores land in one [128, 1024] PSUM tile -> ONE exp.
  - o-matmuls (V with an interleaved ones-column in slot 0 -> row 0 of
    po is the softmax denominator) trail the exp stream by DEPTH tiles.
  - softmax tail: po rows drained to SBUF immediately (frees the PSUM
    bank for the next pair), reciprocal_approx_fast on the denominator
    row (partition 0, in-lane), gpsimd partition-broadcast, one DVE
    multiply. No iterative DVE RECIPROCAL (8 cyc/elem) anywhere.
  - fc_out (K=65 stationary = bf16 ot slices incl. the ones row that
    carries bo), the tail of the V projection (k-tiles 9-15), and the
    Q^T projections for chunks 1-3 are drip-fed into the PE's slack
    inside the exp-bound kt loops via a work queue (~1us items), so the
    PE never idles long enough for the HAM clock gate to re-throttle.
  - the final fc flush runs with a 4-deep psf rotation (borrowing the
    freed score banks), ACT-side drains and PE spacer matmuls to stay
    at the warm clock.
PSUM budget (8 banks): scores 2x[128,1024]=4, po 2x[65,512]=2,
fc/work 2x[128,512]=2.

Inputs are loaded as [E, S] halves with 4KB-contiguous segments (512-
column slab loads were DMA packet-rate bound at 1KB/segment); DMA order
xk -> xv -> xq matches the consumption order of the projection chain,
and the K projection starts on the first half (contraction k-tiles 0-3)
while the second half is still in flight.
"""

import numpy as np
from contextlib import ExitStack

import concourse.tile as tile
from concourse import bacc, mybir
from concourse.bass_utils import run_bass_kernel_spmd

F32R = mybir.dt.float32r
F32 = mybir.dt.float32
BF16 = mybir.dt.bfloat16
AF = mybir.ActivationFunctionType

B, S, E, H, D = 2, 2048, 1024, 16, 64
HL = 4            # heads per core
FL = HL * D       # local feature slice (256)
N_CORES = 8


def build_nc(S=2048, E=1024):
    T = E // 128       # emb k-tiles (8)
    C = S // 512       # 512-wide seq chunks (4)
    QW = 512           # q-chunk width in phase B
    NQC = S // QW      # q chunks (4)
    NKT = S // 128     # key tiles (16)
    DEPTH = 3          # o-matmuls trail exp by this many k-tiles
    scale = 1.0 / (E ** 0.5)

    nc = bacc.Bacc("TRN2", target_bir_lowering=False, debug=False)

    xqT = nc.dram_tensor("xqT", [E, S], BF16, kind="ExternalInput").ap()
    xkT = nc.dram_tensor("xkT", [E, S], BF16, kind="ExternalInput").ap()
    xvT = nc.dram_tensor("xvT", [E, S], BF16, kind="ExternalInput").ap()
    Wq = nc.dram_tensor("Wq", [128, T * 256], BF16, kind="ExternalInput").ap()
    Wk = nc.dram_tensor("Wk", [128, T * 256], BF16, kind="ExternalInput").ap()
    Wv = nc.dram_tensor("Wv", [128, T * 260], BF16, kind="ExternalInput").ap()
    bq = nc.dram_tensor("bq", [1, 256], BF16, kind="ExternalInput").ap()
    bk = nc.dram_tensor("bk", [1, 256], BF16, kind="ExternalInput").ap()
    bv = nc.dram_tensor("bv", [1, 260], BF16, kind="ExternalInput").ap()
    WoT = nc.dram_tensor("WoT", [65, 4 * E], BF16, kind="ExternalInput").ap()
    ones = nc.dram_tensor("ones", [1, 1024], BF16, kind="ExternalInput").ap()
    out = nc.dram_tensor("out", [S, E], BF16, kind="ExternalOutput").ap()

    with tile.TileContext(nc) as tc, ExitStack() as ctx:
        const = ctx.enter_context(tc.tile_pool(name="const", bufs=1))
        persist = ctx.enter_context(tc.tile_pool(name="persist", bufs=1))

        # ---- constants to SBUF ----
        wq_sb = const.tile([128, T * 256], BF16)
        wk_sb = const.tile([128, T * 256], BF16)
        wv_sb = const.tile([128, T * 260], BF16)
        wo_sb = const.tile([65, 4 * E], BF16)
        bq_sb = const.tile([1, 256], BF16)
        bk_sb = const.tile([1, 256], BF16)
        bv_sb = const.tile([1, 260], BF16)
        on_sb = const.tile([1, 1024], BF16)
        # ---- input tensors: [E, S] halves, 4KB-contiguous segments ----
        # Each half holds 4 of the 8 e-row k-tiles: [128, 4*S].
        # DMA order is the critical path: K weights + x first (K-proj
        # heads the serial PE chain), then V, then Q, then Wo.
        xk_h = [persist.tile([128, 4 * S], BF16, tag=f"xk{i}", name=f"xk{i}") for i in range(2)]
        xv_h = [persist.tile([128, 4 * S], BF16, tag=f"xv{i}", name=f"xv{i}") for i in range(2)]
        xq_h = [persist.tile([128, 4 * S], BF16, tag=f"xq{i}", name=f"xq{i}") for i in range(2)]

        def load_x(x_dram, halves):
            src = x_dram.rearrange("(t p) s -> p t s", p=128)
            for i in range(2):
                nc.sync.dma_start(
                    halves[i][:].rearrange("p (t s) -> p t s", s=S),
                    src[:, 4 * i : 4 * i + 4, :],
                )

        nc.sync.dma_start(wk_sb[:], Wk)
        nc.sync.dma_start(bk_sb[:], bk)
        nc.sync.dma_start(on_sb[:], ones)
        load_x(xkT, xk_h)
        nc.sync.dma_start(wv_sb[:], Wv)
        nc.sync.dma_start(bv_sb[:], bv)
        load_x(xvT, xv_h)
        nc.sync.dma_start(wq_sb[:], Wq)
        nc.sync.dma_start(bq_sb[:], bq)
        load_x(xqT, xq_h)
        nc.sync.dma_start(wo_sb[:], WoT)

        # PE warm-up while the first DMAs land (HAM un-throttle needs
        # ~3.4us of busy) + exp/recip table preload.
        with tc.tile_pool(name="wu", bufs=1) as wu_pool, \
             tc.tile_pool(name="wup", bufs=4, space="PSUM") as wup_pool:
            wu = wu_pool.tile([128, 640], BF16)
            nc.gpsimd.memset(wu[:], 0.0)
            wux = wu_pool.tile([1, 32], F32, name="wux")
            nc.scalar.activation(wux[:], wu[0:1, 0:32], AF.Exp, scale=1.0)
            for i in range(24):
                wp = wup_pool.tile([128, 512], F32, tag="wup", name="wup")
                nc.tensor.matmul(wp[:], wu[:, 0:128], wu[:, 128:640],
                                 start=True, stop=True)

        qt_sb = [persist.tile([128, S], BF16, tag=f"qt{j}", name=f"qt{j}")
                 for j in range(2)]
        kt_sb = [persist.tile([128, S], BF16, tag=f"kt{j}", name=f"kt{j}")
                 for j in range(2)]
        v_sb = persist.tile([128, NKT * 260], BF16, tag="v", name="v_sb")

        # PSUM pool used by projection groups: phase-A pools up front,
        # then swapped to the fc pool for filler groups inside phase B
        # (the whole 8-bank budget belongs to phase B by then).
        qk_pool = [None]

        def xslice(halves, t, lo, w):
            i, tt = t // 4, t % 4
            return halves[i][:, tt * S + lo : tt * S + lo + w]

        def qk_part1(x_h, w_sb, j, lo, w, cell):
            """First half of a Q^T/K^T block: contraction tiles 0-3
            (only needs the first DMA'd half of x)."""
            ps = qk_pool[0].tile([128, w], F32, tag="psf", name="psf")
            cell.append(ps)
            for t in range(4):
                nc.tensor.matmul(
                    ps[:],
                    w_sb[:, t * 256 + j * 128 : t * 256 + j * 128 + 128],
                    xslice(x_h, t, lo, w),
                    start=(t == 0), stop=False,
                )

        def qk_part2(x_h, w_sb, b_sb, dst, j, lo, w, cell):
            """Second half: tiles 4-7 + bias row, then drain to SBUF."""
            ps = cell.pop()
            for t in range(4, T):
                nc.tensor.matmul(
                    ps[:],
                    w_sb[:, t * 256 + j * 128 : t * 256 + j * 128 + 128],
                    xslice(x_h, t, lo, w),
                    start=False, stop=False,
                )
            nc.tensor.matmul(
                ps[:], b_sb[:, j * 128 : (j + 1) * 128],
                on_sb[:, 0:w], start=False, stop=True,
            )
            nc.vector.tensor_copy(dst[:, lo : lo + w], ps[:])

        def emit_qk_group(x_h, w_sb, b_sb, dst, j, lo, w):
            cell = []
            qk_part1(x_h, w_sb, j, lo, w, cell)
            qk_part2(x_h, w_sb, b_sb, dst, j, lo, w, cell)

        def v_part1(c, s4, cell):
            ps = qk_pool[0].tile([128, 512], F32, tag="psf", name="psf")
            cell.append(ps)
            for t in range(4):
                nc.tensor.matmul(
                    ps[:, 0:260],
                    xslice(xv_h, t, c * 512 + s4 * 128, 128),
                    wv_sb[:, t * 260 : (t + 1) * 260],
                    start=(t == 0), stop=False,
                )

        def v_part2(c, s4, cell):
            ps = cell.pop()
            for t in range(4, T):
                nc.tensor.matmul(
                    ps[:, 0:260],
                    xslice(xv_h, t, c * 512 + s4 * 128, 128),
                    wv_sb[:, t * 260 : (t + 1) * 260],
                    start=False, stop=False,
                )
            nc.tensor.matmul(
                ps[:, 0:260], on_sb[:, 0:128], bv_sb[:],
                start=False, stop=True,
            )
            nc.vector.tensor_copy(
                v_sb[:, (4 * c + s4) * 260 : (4 * c + s4 + 1) * 260],
                ps[:, 0:260],
            )

        def emit_v_group(c, s4):
            cell = []
            v_part1(c, s4, cell)
            v_part2(c, s4, cell)

        # ---- phase A (upfront part): K^T, V, Q^T chunk 0 ----
        # K first as N=1024 groups: all part1s (need only the first xk
        # half, so the PE starts earlier), then all part2s.
        with tc.tile_pool(name="psA_qk", bufs=8, space="PSUM") as psA_qk:
            qk_pool[0] = psA_qk
            cells = {}
            for c in range(C):
                for j in range(2):
                    cells[(c, j)] = []
                    qk_part1(xk_h, wk_sb, j, c * 512, 512, cells[(c, j)])
            for c in range(C):
                for j in range(2):
                    qk_part2(xk_h, wk_sb, bk_sb, kt_sb[j], j, c * 512,
                             512, cells[(c, j)])
        with tc.tile_pool(name="psA_qk2", bufs=3, space="PSUM") as psA_qk2:
            qk_pool[0] = psA_qk2
            for kt in range(9):
                emit_v_group(kt // 4, kt % 4)
            for j in range(2):
                emit_qk_group(xq_h, wq_sb, bq_sb, qt_sb[j], j, 0, 512)

        # Deferred work queues, drip-fed into phase-B PE slack in ~1us
        # items so they never starve the exp stream.
        filler = []   # V k-tiles 6-15, then remaining Q^T half-groups
        for kt in range(9, NKT):
            c, s4 = kt // 4, kt % 4
            cell = []
            filler.append(lambda c=c, s4=s4, cell=cell: v_part1(c, s4, cell))
            filler.append(lambda c=c, s4=s4, cell=cell: v_part2(c, s4, cell))
        for c in range(1, C):
            for j in range(2):
                cell = []
                filler.append(
                    lambda j=j, c=c, cell=cell:
                    qk_part1(xq_h, wq_sb, j, c * 512, 512, cell))
                filler.append(
                    lambda j=j, c=c, cell=cell:
                    qk_part2(xq_h, wq_sb, bq_sb, qt_sb[j], j, c * 512, 512,
                             cell))
        fc_queue = []  # fc_out half-groups from the previous q-chunk

        def pop_work():
            if filler:
                filler.pop(0)()
            elif fc_queue:
                fc_queue.pop(0)()

        # ---- phase B: attention + fc_out ----
        with tc.tile_pool(name="pt", bufs=DEPTH + 3) as pt_pool, \
             tc.tile_pool(name="raw", bufs=4) as raw_pool, \
             tc.tile_pool(name="rc", bufs=4) as rc_pool, \
             tc.tile_pool(name="bc", bufs=4) as bc_pool, \
             tc.tile_pool(name="ot", bufs=8) as ot_pool, \
             tc.tile_pool(name="os", bufs=2) as os_pool, \
             tc.tile_pool(name="psB_s", bufs=2, space="PSUM") as psB_s, \
             tc.tile_pool(name="psB_o", bufs=1, space="PSUM") as psB_o, \
             tc.tile_pool(name="psB_f", bufs=2, space="PSUM") as psB_f:
            qk_pool[0] = psB_f  # filler projections borrow the fc slots
            drain_on_act = [False]  # final flush: DVE is backlogged, ACT idle

            flush_cnt = [0]

            def emit_fc_half(qc, ss, e2, ots, cell):
                if e2 == 0:
                    cell.append(os_pool.tile([128, E], BF16, tag="osb",
                                             name="osb"))
                o_sb = cell[0]
                if drain_on_act[0]:
                    # final flush: the score banks are free, use them for
                    # a 4-deep psf rotation so matmuls never wait drains
                    flush_cnt[0] += 1
                    if flush_cnt[0] % 2:
                        ps_f = psB_s.tile([128, 1024], F32, tag="pss",
                                          name="pss")[:, 0:512]
                    else:
                        ps_f = psB_f.tile([128, 512], F32, tag="psf",
                                          name="psf")
                else:
                    ps_f = psB_f.tile([128, 512], F32, tag="psf", name="psf")
                for h in range(4):
                    nc.tensor.matmul(
                        ps_f[:],
                        ots[h][:, ss * 128 : (ss + 1) * 128],
                        wo_sb[:, h * E + e2 * 512 : h * E + e2 * 512 + 512],
                        start=(h == 0), stop=(h == 3),
                    )
                if drain_on_act[0]:
                    nc.scalar.copy(o_sb[:, e2 * 512 : (e2 + 1) * 512],
                                   ps_f[:])
                else:
                    nc.vector.tensor_copy(
                        o_sb[:, e2 * 512 : (e2 + 1) * 512], ps_f[:])
                if e2 == 1:
                    nc.sync.dma_start(
                        out[qc * QW + ss * 128 : qc * QW + ss * 128 + 128, :],
                        o_sb[:],
                    )

            for qc in range(NQC):
                ot_tiles = {}
                for j in range(2):
                    po = [psB_o.tile([65, QW], F32, tag=f"po{e}", name=f"po{e}")
                          for e in range(2)]
                    pts = {}

                    def emit_o(kt, po=po, pts=pts):
                        for e in range(2):
                            nc.tensor.matmul(
                                po[e][:],
                                v_sb[:, kt * 260 + 65 * (2 * j + e)
                                     : kt * 260 + 65 * (2 * j + e) + 65],
                                pts[kt][:, e * 512 : (e + 1) * 512],
                                start=(kt == 0), stop=(kt == NKT - 1),
                            )
                        del pts[kt]

                    for kt in range(NKT):
                        ps_s = psB_s.tile([128, 1024], F32, tag="pss",
                                          name="pss")
                        for e in range(2):
                            nc.tensor.matmul(
                                ps_s[:, e * 512 : (e + 1) * 512],
                                kt_sb[j][64 * e : 64 * e + 64,
                                         kt * 128 : (kt + 1) * 128],
                                qt_sb[j][64 * e : 64 * e + 64,
                                         qc * QW : qc * QW + QW],
                                start=True, stop=True,
                            )
                        pt = pt_pool.tile([128, 1024], BF16, tag="pt",
                                          name="pt")
                        nc.scalar.activation(pt[:], ps_s[:], AF.Exp,
                                             scale=scale)
                        pts[kt] = pt
                        if kt >= DEPTH:
                            emit_o(kt - DEPTH)
                        if filler:
                            pop_work()
                        elif kt >= 5 and kt % 2 == 1:
                            pop_work()
                    for kt in range(NKT - DEPTH, NKT):
                        emit_o(kt)

                    if qc == NQC - 1 and j == 1:
                        # Keep the PE busy across the final softmax tail
                        # so the last fc flush runs at the warm clock.
                        for i in range(16):
                            ka = psB_s.tile([128, 1024], F32, tag="pss",
                                            name="pss")
                            nc.tensor.matmul(
                                ka[:, 0:512], kt_sb[0][0:64, 0:128],
                                qt_sb[0][0:64, 0:512],
                                start=True, stop=True,
                            )

                    # softmax tail: drain po to SBUF immediately (frees
                    # the PSUM banks for the next pair), then recip the
                    # denominator rows, broadcast, multiply.
                    raws = []
                    for e in range(2):
                        raw = raw_pool.tile([65, QW], F32, tag="raw",
                                            name="raw")
                        nc.vector.tensor_copy(raw[:], po[e][:])
                        raws.append(raw)
                    rcs = []
                    for e in range(2):
                        rc = rc_pool.tile([1, QW], F32, tag="rc", name="rc")
                        with nc.allow_low_precision(reason="softmax denom"):
                            nc.vector.reciprocal_approx_fast(
                                rc[:], raws[e][0:1, :])
                        rcs.append(rc)
                    bcs = []
                    for e in range(2):
                        bc = bc_pool.tile([65, QW], F32, tag="bc", name="bc")
                        nc.gpsimd.partition_broadcast(bc[:], rcs[e][:])
                        bcs.append(bc)
                    for e in range(2):
                        ot = ot_pool.tile([65, QW], BF16, tag="ot", name="ot")
                        nc.vector.tensor_mul(ot[:], raws[e][:], bcs[e][:])
                        ot_tiles[2 * j + e] = ot

                for ss in range(QW // 128):
                    cell = []
                    for e2 in range(2):
                        fc_queue.append(
                            lambda q=qc, s=ss, e2=e2, o=dict(ot_tiles),
                                   cell=cell:
                            emit_fc_half(q, s, e2, o, cell))
            drain_on_act[0] = True
            while fc_queue:
                pop_work()
                # spacer keeps the PE from idling into a HAM re-throttle
                # while the next psf slot drains
                ka = psB_s.tile([128, 1024], F32, tag="pss", name="pss")
                nc.tensor.matmul(ka[:, 0:512], kt_sb[0][0:64, 0:128],
                                 qt_sb[0][0:64, 0:512],
                                 start=True, stop=True)

    nc.compile()
    return nc


_NC_CACHE = [None]


def _get_nc():
    if _NC_CACHE[0] is None:
        _NC_CACHE[0] = build_nc(S=S, E=E)
    return _NC_CACHE[0]


def _pack_w(W):
    """[E, F] -> [128, (E//128)*F], one 128-row k-tile after another,
    so the SBUF weight DMA is fully contiguous."""
    E_, F_ = W.shape
    T_ = E_ // 128
    return np.ascontiguousarray(
        W.reshape(T_, 128, F_).transpose(1, 0, 2).reshape(128, T_ * F_))


def make_in_maps(query, key, value, Wq, bq, Wk, bk, Wv, bv, Wo, bo):
    """Shard the full inputs into the 8 per-core input dicts."""
    bf = mybir.dt.np(BF16)
    f32 = np.float32
    query = np.asarray(query, f32)
    key = np.asarray(key, f32)
    value = np.asarray(value, f32)
    Wq, bq = np.asarray(Wq, f32), np.asarray(bq, f32)
    Wk, bk = np.asarray(Wk, f32), np.asarray(bk, f32)
    Wv, bv = np.asarray(Wv, f32), np.asarray(bv, f32)
    Wo, bo = np.asarray(Wo, f32), np.asarray(bo, f32)

    xT = {}
    for b in range(B):
        xT[b] = (
            np.ascontiguousarray(query[b].T).astype(bf),
            np.ascontiguousarray(key[b].T).astype(bf),
            np.ascontiguousarray(value[b].T).astype(bf),
        )

    ones = np.ones((1, 1024), bf)
    in_maps = []
    for c in range(N_CORES):
        b, g = c // 4, c % 4
        fs = slice(FL * g, FL * g + FL)
        # projection weights: reference computes x @ W.T, so the device
        # weight matrix is W.T's column slice = W[fs, :].T  [E, FL]
        wq_c = np.ascontiguousarray(Wq[fs, :].T)
        wk_c = np.ascontiguousarray(Wk[fs, :].T)
        wv_c = np.ascontiguousarray(Wv[fs, :].T)
        # V with interleaved ones-columns (via the bias row)
        wv_pack = np.zeros((E, HL * 65), f32)
        bv_pack = np.zeros((1, HL * 65), f32)
        bq_c = bq[fs][None, :]
        bk_c = bk[fs][None, :]
        for h in range(HL):
            # ones column FIRST: the softmax denominator lands on
            # partition 0 of po, where the DVE recip can read it in-lane
            bv_pack[0, 65 * h] = 1.0
            wv_pack[:, 65 * h + 1 : 65 * h + 65] = wv_c[:, 64 * h : 64 * h + 64]
            bv_pack[0, 65 * h + 1 : 65 * h + 65] = bv[fs][64 * h : 64 * h + 64]
        # fc_out rows for the local features; row 64 of head-slot 0
        # carries bo on one core per batch group (the normalized
        # ones-row multiplies it)
        wot = np.zeros((65, HL * E), f32)
        for h in range(HL):
            wot[1:65, E * h : E * h + E] = Wo[:, FL * g + 64 * h : FL * g + 64 * h + 64].T
        if g == 0:
            wot[0, 0:E] = bo
        in_maps.append({
            "xqT": xT[b][0], "xkT": xT[b][1], "xvT": xT[b][2],
            "Wq": _pack_w(wq_c).astype(bf),
            "Wk": _pack_w(wk_c).astype(bf),
            "Wv": _pack_w(wv_pack).astype(bf),
            "bq": bq_c.astype(bf), "bk": bk_c.astype(bf),
            "bv": bv_pack.astype(bf),
            "WoT": wot.astype(bf),
            "ones": ones,
        })
    return in_maps


def assemble_output(results):
    """Sum the row-parallel partial fc_out results per batch."""
    out = np.empty((B, S, E), np.float32)
    for b in range(B):
        acc = results[4 * b]["out"].astype(np.float32).copy()
        for g in range(1, 4):
            acc += results[4 * b + g]["out"]
        out[b] = acc
    return out


def kernel(query, key, value, Wq, bq, Wk, bk, Wv, bv, Wo, bo, **run_kwargs):
    nc = _get_nc()
    in_maps = make_in_maps(query, key, value, Wq, bq, Wk, bk, Wv, bv, Wo, bo)
    res = run_bass_kernel_spmd(nc, in_maps, core_ids=list(range(N_CORES)),
                               **run_kwargs)
    out = assemble_output(res.results)
    kernel.last_result = res
    return out


# revision 39
# speedup vs baseline: 1.0088x; 1.0088x over previous
"""Trainium2 Bass kernel for nn_MultiHeadAttention (B=2, S=2048, E=1024,
H=16, D=64) on 8 NeuronCores.

Sharding: core c -> (batch b = c//4, head-group g = c%4). Each core
computes Q/K/V projections for its batch restricted to its 4 heads
(column-parallel Wq/Wk/Wv), full attention for those heads, and a
row-parallel partial fc_out (the 256 local features of Wo, bf16). The
host sums the 4 bf16 partial outputs per batch in fp32; the fc bias bo
is folded into one core per batch group via the normalized ones-row.

The ScalarE exp stream is the roofline: 128 ACTIVATEs of [128, 1024]
(~1.1us each) that everything else must hide behind.
  - q is processed in 512-wide chunks; heads in pairs (j in {0,1}) whose
    K^T/Q^T live on partition halves 0-63 / 64-127 of kt_sb[j]/qt_sb[j].
  - s-matmuls for the two heads of a pair are emitted back-to-back with
    K=64 stationaries at base partitions 0 and 64: the PE runs them
    CONCURRENTLY as row-tiles (2 MMs per N=512 slot when warm).
  - both heads' scores land in one [128, 1024] PSUM tile -> ONE exp.
  - o-matmuls (V with an interleaved ones-column in slot 0 -> row 0 of
    po is the softmax denominator) trail the exp stream by DEPTH tiles.
  - softmax tail: po rows drained to SBUF immediately (frees the PSUM
    bank for the next pair), reciprocal_approx_fast on the denominator
    row (partition 0, in-lane), gpsimd partition-broadcast, one DVE
    multiply. No iterative DVE RECIPROCAL (8 cyc/elem) anywhere.
  - fc_out (K=65 stationary = bf16 ot slices incl. the ones row that
    carries bo), the tail of the V projection (k-tiles 9-15), and the
    Q^T projections for chunks 1-3 are drip-fed into the PE's slack
    inside the exp-bound kt loops via a work queue (~1us items), so the
    PE never idles long enough for the HAM clock gate to re-throttle.
  - the final fc flush runs with a 4-deep psf rotation (borrowing the
    freed score banks), ACT-side drains and PE spacer matmuls to stay
    at the warm clock.
PSUM budget (8 banks): scores 2x[128,1024]=4, po 2x[65,512]=2,
fc/work 2x[128,512]=2.

Inputs are loaded as [E, S] halves with 4KB-contiguous segments (512-
column slab loads were DMA packet-rate bound at 1KB/segment); DMA order
xk -> xv -> xq matches the consumption order of the projection chain,
and the K projection starts on the first half (contraction k-tiles 0-3)
while the second half is still in flight.
"""

import numpy as np
from contextlib import ExitStack

import concourse.tile as tile
from concourse import bacc, mybir
from concourse.bass_utils import run_bass_kernel_spmd

F32R = mybir.dt.float32r
F32 = mybir.dt.float32
BF16 = mybir.dt.bfloat16
AF = mybir.ActivationFunctionType

B, S, E, H, D = 2, 2048, 1024, 16, 64
HL = 4            # heads per core
FL = HL * D       # local feature slice (256)
N_CORES = 8


def build_nc(S=2048, E=1024):
    T = E // 128       # emb k-tiles (8)
    C = S // 512       # 512-wide seq chunks (4)
    QW = 512           # q-chunk width in phase B
    NQC = S // QW      # q chunks (4)
    NKT = S // 128     # key tiles (16)
    DEPTH = 3          # o-matmuls trail exp by this many k-tiles
    scale = 1.0 / (E ** 0.5)

    nc = bacc.Bacc("TRN2", target_bir_lowering=False, debug=False)

    xqT = nc.dram_tensor("xqT", [E, S], BF16, kind="ExternalInput").ap()
    xkT = nc.dram_tensor("xkT", [E, S], BF16, kind="ExternalInput").ap()
    xvT = nc.dram_tensor("xvT", [E, S], BF16, kind="ExternalInput").ap()
    Wq = nc.dram_tensor("Wq", [128, T * 256], BF16, kind="ExternalInput").ap()
    Wk = nc.dram_tensor("Wk", [128, T * 256], BF16, kind="ExternalInput").ap()
    Wv = nc.dram_tensor("Wv", [128, T * 260], BF16, kind="ExternalInput").ap()
    bq = nc.dram_tensor("bq", [1, 256], BF16, kind="ExternalInput").ap()
    bk = nc.dram_tensor("bk", [1, 256], BF16, kind="ExternalInput").ap()
    bv = nc.dram_tensor("bv", [1, 260], BF16, kind="ExternalInput").ap()
    WoT = nc.dram_tensor("WoT", [65, 4 * E], BF16, kind="ExternalInput").ap()
    ones = nc.dram_tensor("ones", [1, 1024], BF16, kind="ExternalInput").ap()
    out = nc.dram_tensor("out", [S, E], BF16, kind="ExternalOutput").ap()

    with tile.TileContext(nc) as tc, ExitStack() as ctx:
        const = ctx.enter_context(tc.tile_pool(name="const", bufs=1))
        persist = ctx.enter_context(tc.tile_pool(name="persist", bufs=1))

        # ---- constants to SBUF ----
        wq_sb = const.tile([128, T * 256], BF16)
        wk_sb = const.tile([128, T * 256], BF16)
        wv_sb = const.tile([128, T * 260], BF16)
        wo_sb = const.tile([65, 4 * E], BF16)
        bq_sb = const.tile([1, 256], BF16)
        bk_sb = const.tile([1, 256], BF16)
        bv_sb = const.tile([1, 260], BF16)
        on_sb = const.tile([1, 1024], BF16)
        # ---- input tensors: [E, S] halves, 4KB-contiguous segments ----
        # Each half holds 4 of the 8 e-row k-tiles: [128, 4*S].
        # DMA order is the critical path: K weights + x first (K-proj
        # heads the serial PE chain), then V, then Q, then Wo.
        xk_h = [persist.tile([128, 4 * S], BF16, tag=f"xk{i}", name=f"xk{i}") for i in range(2)]
        xv_h = [persist.tile([128, 4 * S], BF16, tag=f"xv{i}", name=f"xv{i}") for i in range(2)]
        xq_h = [persist.tile([128, 4 * S], BF16, tag=f"xq{i}", name=f"xq{i}") for i in range(2)]

        def load_x(x_dram, halves):
            src = x_dram.rearrange("(t p) s -> p t s", p=128)
            for i in range(2):
                nc.sync.dma_start(
                    halves[i][:].rearrange("p (t s) -> p t s", s=S),
                    src[:, 4 * i : 4 * i + 4, :],
                )

        nc.sync.dma_start(wk_sb[:], Wk)
        nc.sync.dma_start(bk_sb[:], bk)
        nc.sync.dma_start(on_sb[:], ones)
        load_x(xkT, xk_h)
        nc.sync.dma_start(wv_sb[:], Wv)
        nc.sync.dma_start(bv_sb[:], bv)
        load_x(xvT, xv_h)
        nc.sync.dma_start(wq_sb[:], Wq)
        nc.sync.dma_start(bq_sb[:], bq)
        load_x(xqT, xq_h)
        nc.sync.dma_start(wo_sb[:], WoT)

        # PE warm-up while the first DMAs land (HAM un-throttle needs
        # ~3.4us of busy) + exp/recip table preload.
        with tc.tile_pool(name="wu", bufs=1) as wu_pool, \
             tc.tile_pool(name="wup", bufs=4, space="PSUM") as wup_pool:
            wu = wu_pool.tile([128, 640], BF16)
            nc.gpsimd.memset(wu[:], 0.0)
            wux = wu_pool.tile([1, 32], F32, name="wux")
            nc.scalar.activation(wux[:], wu[0:1, 0:32], AF.Exp, scale=1.0)
            for i in range(24):
                wp = wup_pool.tile([128, 512], F32, tag="wup", name="wup")
                nc.tensor.matmul(wp[:], wu[:, 0:128], wu[:, 128:640],
                                 start=True, stop=True)

        qt_sb = [persist.tile([128, S], BF16, tag=f"qt{j}", name=f"qt{j}")
                 for j in range(2)]
        kt_sb = [persist.tile([128, S], BF16, tag=f"kt{j}", name=f"kt{j}")
                 for j in range(2)]
        v_sb = persist.tile([128, NKT * 260], BF16, tag="v", name="v_sb")

        # PSUM pool used by projection groups: phase-A pools up front,
        # then swapped to the fc pool for filler groups inside phase B
        # (the whole 8-bank budget belongs to phase B by then).
        qk_pool = [None]

        def xslice(halves, t, lo, w):
            i, tt = t // 4, t % 4
            return halves[i][:, tt * S + lo : tt * S + lo + w]

        def qk_part1(x_h, w_sb, j, lo, w, cell):
            """First half of a Q^T/K^T block: contraction tiles 0-3
            (only needs the first DMA'd half of x)."""
            ps = qk_pool[0].tile([128, w], F32, tag="psf", name="psf")
            cell.append(ps)
            for t in range(4):
                nc.tensor.matmul(
                    ps[:],
                    w_sb[:, t * 256 + j * 128 : t * 256 + j * 128 + 128],
                    xslice(x_h, t, lo, w),
                    start=(t == 0), stop=False,
                )

        def qk_part2(x_h, w_sb, b_sb, dst, j, lo, w, cell):
            """Second half: tiles 4-7 + bias row, then drain to SBUF."""
            ps = cell.pop()
            for t in range(4, T):
                nc.tensor.matmul(
                    ps[:],
                    w_sb[:, t * 256 + j * 128 : t * 256 + j * 128 + 128],
                    xslice(x_h, t, lo, w),
                    start=False, stop=False,
                )
            nc.tensor.matmul(
                ps[:], b_sb[:, j * 128 : (j + 1) * 128],
                on_sb[:, 0:w], start=False, stop=True,
            )
            nc.vector.tensor_copy(dst[:, lo : lo + w], ps[:])

        def emit_qk_group(x_h, w_sb, b_sb, dst, j, lo, w):
            cell = []
            qk_part1(x_h, w_sb, j, lo, w, cell)
            qk_part2(x_h, w_sb, b_sb, dst, j, lo, w, cell)

        def v_part1(c, s4, cell):
            ps = qk_pool[0].tile([128, 512], F32, tag="psf", name="psf")
            cell.append(ps)
            for t in range(4):
                nc.tensor.matmul(
                    ps[:, 0:260],
                    xslice(xv_h, t, c * 512 + s4 * 128, 128),
                    wv_sb[:, t * 260 : (t + 1) * 260],
                    start=(t == 0), stop=False,
                )

        def v_part2(c, s4, cell):
            ps = cell.pop()
            for t in range(4, T):
                nc.tensor.matmul(
                    ps[:, 0:260],
                    xslice(xv_h, t, c * 512 + s4 * 128, 128),
                    wv_sb[:, t * 260 : (t + 1) * 260],
                    start=False, stop=False,
                )
            nc.tensor.matmul(
                ps[:, 0:260], on_sb[:, 0:128], bv_sb[:],
                start=False, stop=True,
            )
            nc.vector.tensor_copy(
                v_sb[:, (4 * c + s4) * 260 : (4 * c + s4 + 1) * 260],
                ps[:, 0:260],
            )

        def emit_v_group(c, s4):
            cell = []
            v_part1(c, s4, cell)
            v_part2(c, s4, cell)

        # ---- phase A (upfront part): K^T, V, Q^T chunk 0 ----
        # K first as N=1024 groups: all part1s (need only the first xk
        # half, so the PE starts earlier), then all part2s.
        with tc.tile_pool(name="psA_qk", bufs=8, space="PSUM") as psA_qk:
            qk_pool[0] = psA_qk
            cells = {}
            for c in range(C):
                for j in range(2):
                    cells[(c, j)] = []
                    qk_part1(xk_h, wk_sb, j, c * 512, 512, cells[(c, j)])
            for c in range(C):
                for j in range(2):
                    qk_part2(xk_h, wk_sb, bk_sb, kt_sb[j], j, c * 512,
                             512, cells[(c, j)])
        with tc.tile_pool(name="psA_qk2", bufs=3, space="PSUM") as psA_qk2:
            qk_pool[0] = psA_qk2
            for kt in range(9):
                emit_v_group(kt // 4, kt % 4)
            for j in range(2):
                emit_qk_group(xq_h, wq_sb, bq_sb, qt_sb[j], j, 0, 512)

        # Deferred work queues, drip-fed into phase-B PE slack in ~1us
        # items so they never starve the exp stream.
        filler = []   # V k-tiles 6-15, then remaining Q^T half-groups
        for kt in range(9, NKT):
            c, s4 = kt // 4, kt % 4
            cell = []
            filler.append(lambda c=c, s4=s4, cell=cell: v_part1(c, s4, cell))
            filler.append(lambda c=c, s4=s4, cell=cell: v_part2(c, s4, cell))
        for c in range(1, C):
            for j in range(2):
                cell = []
                filler.append(
                    lambda j=j, c=c, cell=cell:
                    qk_part1(xq_h, wq_sb, j, c * 512, 512, cell))
                filler.append(
                    lambda j=j, c=c, cell=cell:
                    qk_part2(xq_h, wq_sb, bq_sb, qt_sb[j], j, c * 512, 512,
                             cell))
        fc_queue = []  # fc_out half-groups from the previous q-chunk

        def pop_work():
            if filler:
                filler.pop(0)()
            elif fc_queue:
                fc_queue.pop(0)()

        # ---- phase B: attention + fc_out ----
        with tc.tile_pool(name="pt", bufs=DEPTH + 3) as pt_pool, \
             tc.tile_pool(name="raw", bufs=4) as raw_pool, \
             tc.tile_pool(name="rc", bufs=4) as rc_pool, \
             tc.tile_pool(name="bc", bufs=4) as bc_pool, \
             tc.tile_pool(name="ot", bufs=8) as ot_pool, \
             tc.tile_pool(name="os", bufs=2) as os_pool, \
             tc.tile_pool(name="psB_s", bufs=2, space="PSUM") as psB_s, \
             tc.tile_pool(name="psB_o", bufs=1, space="PSUM") as psB_o, \
             tc.tile_pool(name="psB_f", bufs=2, space="PSUM") as psB_f:
            qk_pool[0] = psB_f  # filler projections borrow the fc slots
            drain_on_act = [False]  # final flush: DVE is backlogged, ACT idle

            flush_cnt = [0]

            def emit_fc_half(qc, ss, e2, ots, cell):
                if e2 == 0:
                    cell.append(os_pool.tile([128, E], BF16, tag="osb",
                                             name="osb"))
                o_sb = cell[0]
                if drain_on_act[0]:
                    # final flush: the score banks are free, use them for
                    # a 4-deep psf rotation so matmuls never wait drains
                    flush_cnt[0] += 1
                    if flush_cnt[0] % 2:
                        ps_f = psB_s.tile([128, 1024], F32, tag="pss",
                                          name="pss")[:, 0:512]
                    else:
                        ps_f = psB_f.tile([128, 512], F32, tag="psf",
                                          name="psf")
                else:
                    ps_f = psB_f.tile([128, 512], F32, tag="psf", name="psf")
                for h in range(4):
                    nc.tensor.matmul(
                        ps_f[:],
                        ots[h][:, ss * 128 : (ss + 1) * 128],
                        wo_sb[:, h * E + e2 * 512 : h * E + e2 * 512 + 512],
                        start=(h == 0), stop=(h == 3),
                    )
                if drain_on_act[0]:
                    nc.scalar.copy(o_sb[:, e2 * 512 : (e2 + 1) * 512],
                                   ps_f[:])
                else:
                    nc.vector.tensor_copy(
                        o_sb[:, e2 * 512 : (e2 + 1) * 512], ps_f[:])
                if e2 == 1:
                    nc.sync.dma_start(
                        out[qc * QW + ss * 128 : qc * QW + ss * 128 + 128, :],
                        o_sb[:],
                    )

            def make_tail(po, j, qc, ot_tiles):
                """Softmax tail for one pair: drain po to SBUF (frees the
                PSUM banks), recip the denominator rows, broadcast,
                multiply; after pair j=1, enqueue the q-chunk's fc."""
                def tail():
                    raws = []
                    for e in range(2):
                        raw = raw_pool.tile([65, QW], F32, tag="raw",
                                            name="raw")
                        nc.vector.tensor_copy(raw[:], po[e][:])
                        raws.append(raw)
                    rcs = []
                    for e in range(2):
                        rc = rc_pool.tile([1, QW], F32, tag="rc", name="rc")
                        with nc.allow_low_precision(reason="softmax denom"):
                            nc.vector.reciprocal_approx_fast(
                                rc[:], raws[e][0:1, :])
                        rcs.append(rc)
                    bcs = []
                    for e in range(2):
                        bc = bc_pool.tile([65, QW], F32, tag="bc", name="bc")
                        nc.gpsimd.partition_broadcast(bc[:], rcs[e][:])
                        bcs.append(bc)
                    for e in range(2):
                        ot = ot_pool.tile([65, QW], BF16, tag="ot",
                                          name="ot")
                        nc.vector.tensor_mul(ot[:], raws[e][:], bcs[e][:])
                        ot_tiles[2 * j + e] = ot
                    if j == 1:
                        for ss in range(QW // 128):
                            cell = []
                            for e2 in range(2):
                                fc_queue.append(
                                    lambda q=qc, s=ss, e2=e2,
                                           o=ot_tiles, cell=cell:
                                    emit_fc_half(q, s, e2, o, cell))
                return tail

            # The trailing DEPTH o-matmuls of a pair can only run after
            # its final exps, which would delay the NEXT pair's first
            # s/exp by ~2-3us at every boundary. Instead they (and the
            # softmax tail) are carried over and emitted inside the next
            # pair's first iterations, behind its s/exp.
            pending = [None]   # (emit_o, [remaining kts], tail_fn)

            for qc in range(NQC):
                ot_tiles = {}
                for j in range(2):
                    po = [psB_o.tile([65, QW], F32, tag=f"po{e}", name=f"po{e}")
                          for e in range(2)]
                    pts = {}

                    def emit_o(kt, po=po, pts=pts, j=j):
                        for e in range(2):
                            nc.tensor.matmul(
                                po[e][:],
                                v_sb[:, kt * 260 + 65 * (2 * j + e)
                                     : kt * 260 + 65 * (2 * j + e) + 65],
                                pts[kt][:, e * 512 : (e + 1) * 512],
                                start=(kt == 0), stop=(kt == NKT - 1),
                            )
                        del pts[kt]

                    for kt in range(NKT):
                        ps_s = psB_s.tile([128, 1024], F32, tag="pss",
                                          name="pss")
                        for e in range(2):
                            nc.tensor.matmul(
                                ps_s[:, e * 512 : (e + 1) * 512],
                                kt_sb[j][64 * e : 64 * e + 64,
                                         kt * 128 : (kt + 1) * 128],
                                qt_sb[j][64 * e : 64 * e + 64,
                                         qc * QW : qc * QW + QW],
                                start=True, stop=True,
                            )
                        pt = pt_pool.tile([128, 1024], BF16, tag="pt",
                                          name="pt")
                        nc.scalar.activation(pt[:], ps_s[:], AF.Exp,
                                             scale=scale)
                        pts[kt] = pt
                        if pending[0] is not None:
                            p_o, p_kts, p_tail = pending[0]
                            p_o(p_kts.pop(0))
                            if len(p_kts) == 1:
                                pass
                            elif not p_kts:
                                p_tail()
                                pending[0] = None
                        if kt >= DEPTH:
                            emit_o(kt - DEPTH)
                        if filler:
                            pop_work()
                        elif kt >= 5 and kt % 2 == 1:
                            pop_work()
                    pending[0] = (emit_o,
                                  list(range(NKT - DEPTH, NKT)),
                                  make_tail(po, j, qc, ot_tiles))

            # final pair: trailing o's + tail, then keep the PE busy so
            # the fc flush runs at the warm clock.
            p_o, p_kts, p_tail = pending[0]
            for kt in p_kts:
                p_o(kt)
            p_tail()
            for i in range(24):
                ka = psB_s.tile([128, 1024], F32, tag="pss", name="pss")
                nc.tensor.matmul(ka[:, 0:512], kt_sb[0][0:64, 0:128],
                                 qt_sb[0][0:64, 0:512],
                                 start=True, stop=True)
            drain_on_act[0] = True
            while fc_queue:
                pop_work()
                # spacer keeps the PE from idling into a HAM re-throttle
                # while the next psf slot drains
                ka = psB_s.tile([128, 1024], F32, tag="pss", name="pss")
                nc.tensor.matmul(ka[:, 0:512], kt_sb[0][0:64, 0:128],
                                 qt_sb[0][0:64, 0:512],
                                 start=True, stop=True)

    nc.compile()
    return nc


_NC_CACHE = [None]


def _get_nc():
    if _NC_CACHE[0] is None:
        _NC_CACHE[0] = build_nc(S=S, E=E)
    return _NC_CACHE[0]


def _pack_w(W):
    """[E, F] -> [128, (E//128)*F], one 128-row k-tile after another,
    so the SBUF weight DMA is fully contiguous."""
    E_, F_ = W.shape
    T_ = E_ // 128
    return np.ascontiguousarray(
        W.reshape(T_, 128, F_).transpose(1, 0, 2).reshape(128, T_ * F_))


def make_in_maps(query, key, value, Wq, bq, Wk, bk, Wv, bv, Wo, bo):
    """Shard the full inputs into the 8 per-core input dicts."""
    bf = mybir.dt.np(BF16)
    f32 = np.float32
    query = np.asarray(query, f32)
    key = np.asarray(key, f32)
    value = np.asarray(value, f32)
    Wq, bq = np.asarray(Wq, f32), np.asarray(bq, f32)
    Wk, bk = np.asarray(Wk, f32), np.asarray(bk, f32)
    Wv, bv = np.asarray(Wv, f32), np.asarray(bv, f32)
    Wo, bo = np.asarray(Wo, f32), np.asarray(bo, f32)

    xT = {}
    for b in range(B):
        xT[b] = (
            np.ascontiguousarray(query[b].T).astype(bf),
            np.ascontiguousarray(key[b].T).astype(bf),
            np.ascontiguousarray(value[b].T).astype(bf),
        )

    ones = np.ones((1, 1024), bf)
    in_maps = []
    for c in range(N_CORES):
        b, g = c // 4, c % 4
        fs = slice(FL * g, FL * g + FL)
        # projection weights: reference computes x @ W.T, so the device
        # weight matrix is W.T's column slice = W[fs, :].T  [E, FL]
        wq_c = np.ascontiguousarray(Wq[fs, :].T)
        wk_c = np.ascontiguousarray(Wk[fs, :].T)
        wv_c = np.ascontiguousarray(Wv[fs, :].T)
        # V with interleaved ones-columns (via the bias row)
        wv_pack = np.zeros((E, HL * 65), f32)
        bv_pack = np.zeros((1, HL * 65), f32)
        bq_c = bq[fs][None, :]
        bk_c = bk[fs][None, :]
        for h in range(HL):
            # ones column FIRST: the softmax denominator lands on
            # partition 0 of po, where the DVE recip can read it in-lane
            bv_pack[0, 65 * h] = 1.0
            wv_pack[:, 65 * h + 1 : 65 * h + 65] = wv_c[:, 64 * h : 64 * h + 64]
            bv_pack[0, 65 * h + 1 : 65 * h + 65] = bv[fs][64 * h : 64 * h + 64]
        # fc_out rows for the local features; row 64 of head-slot 0
        # carries bo on one core per batch group (the normalized
        # ones-row multiplies it)
        wot = np.zeros((65, HL * E), f32)
        for h in range(HL):
            wot[1:65, E * h : E * h + E] = Wo[:, FL * g + 64 * h : FL * g + 64 * h + 64].T
        if g == 0:
            wot[0, 0:E] = bo
        in_maps.append({
            "xqT": xT[b][0], "xkT": xT[b][1], "xvT": xT[b][2],
            "Wq": _pack_w(wq_c).astype(bf),
            "Wk": _pack_w(wk_c).astype(bf),
            "Wv": _pack_w(wv_pack).astype(bf),
            "bq": bq_c.astype(bf), "bk": bk_c.astype(bf),
            "bv": bv_pack.astype(bf),
            "WoT": wot.astype(bf),
            "ones": ones,
        })
    return in_maps


def assemble_output(results):
    """Sum the row-parallel partial fc_out results per batch."""
    out = np.empty((B, S, E), np.float32)
    for b in range(B):
        acc = results[4 * b]["out"].astype(np.float32).copy()
        for g in range(1, 4):
            acc += results[4 * b + g]["out"]
        out[b] = acc
    return out


def kernel(query, key, value, Wq, bq, Wk, bk, Wv, bv, Wo, bo, **run_kwargs):
    nc = _get_nc()
    in_maps = make_in_maps(query, key, value, Wq, bq, Wk, bk, Wv, bv, Wo, bo)
    res = run_bass_kernel_spmd(nc, in_maps, core_ids=list(range(N_CORES)),
                               **run_kwargs)
    out = assemble_output(res.results)
    kernel.last_result = res
    return out
